# revision 2
# baseline (speedup 1.0000x reference)
"""Trainium2 Bass kernel for nn_DenoisingNet (GNN message passing), v2.

Strategy
--------
The per-edge attention MLP decomposes into per-node scalars:
    log_alpha[e] = a1[row[e]] + a2[col[e]] + b_att
so the MLP runs over N=50k nodes instead of E=800k edges.

Sharding: edges bucketed by row-range (core c owns rows [c*6250,
(c+1)*6250)), row-sorted, whole nodes packed per SBUF partition with a
64-node/partition cap.  Nodes are RE-INDEXED device-side in packed
order: node (p, k) -> local id p*64+k (host permutes x rows), so node
tables are plain [128, 64] SBUF tiles / [8192] DRAM vectors with static
access patterns - no boundary gathers, no scan.

Per-edge col gathers use the MoE `dma_gather` ucode (single_packet=False;
the single-packet path dies above ~1k indices): global node tables are
AllGathered to [65536], widened on-device to [32768, 64] f32 rows (256 B,
two nodes per row, 32 copies each), gathered per 6912-index chunk with
wrapped int16 indices, and a predicated select picks the odd/even half.

Row-side expansions (a1[row], dinv[row]) and the per-node rowsum use
KMAX=64 select-accumulate / select-reduce passes over the [128, F] slot
grid (seg[p,j] = k of the owning node, constant per contiguous run).
"""

import functools

import numpy as np

import concourse.bacc as bacc
import concourse.tile as tile
from concourse import mybir
from concourse.bass_utils import run_bass_kernel_spmd
from concourse.tile import add_dep_helper

# ---- problem constants (hardcoded per contest contract) ----
N = 50000
E = 800000
D = 256
H = 128
CORES = 8
NPC = N // CORES          # 6250 nodes per core
KMAX = 64                 # max packed nodes per edge-partition
LNP = 128 * KMAX          # 8192: local node table length (packed ids)
NT = LNP // 128           # 64 node tiles
FULLN = CORES * LNP       # 65536 global table length
TROWS = FULLN // 2        # 32768 wide-table rows (2 nodes/row)
F = 864                   # edge slots per partition
CH = 54                   # gather chunk columns (2 in-flight chunks stay
                          # under the 1024-desc/lane SWDGE ring)
NCHUNK = F // CH          # chunks (F must be divisible by CH)
NI = 128 * CH             # indices per chunk
IW = NI // 16             # wrapped-idx columns per chunk
ES = 64                   # wide-table row: 64 f32 = 256 B
BF16 = 1

GAMMA = -0.5
ZETA = 1.1
DEBUG_VAR = 1e-07
AOFF = 64.0               # positivity offset for the a1 expansion

f32 = mybir.dt.float32
bf16 = mybir.dt.bfloat16
i32 = mybir.dt.int32
i16 = mybir.dt.int16
xdt = bf16 if BF16 else f32
AF = mybir.ActivationFunctionType
OP = mybir.AluOpType


# ======================================================================
# host-side sharding
# ======================================================================

def _pack_nodes(deg, ne):
    """Sequential greedy pack of nodes (in id order) into 128 partitions.

    Returns part_of_node, k_of_node."""
    part = np.zeros(NPC, np.int32)
    kk = np.zeros(NPC, np.int32)
    p = 0
    slots_p = 0
    k_p = 0
    placed = 0
    target = -(-ne // 128)
    for l in range(NPC):
        d = int(deg[l])
        if p < 127 and (slots_p + d > target or k_p >= KMAX):
            p += 1
            slots_p = 0
            k_p = 0
            target = -(-(ne - placed) // (128 - p))
        assert slots_p + d <= F, f"partition overflow {slots_p}+{d}"
        part[l] = p
        kk[l] = k_p
        slots_p += d
        k_p += 1
        placed += d
    return part, kk


def _wrap16(arr):
    """[128, F] idx grid -> wrapped int16 [128, NCHUNK*IW] for dma_gather."""
    out = np.empty((128, NCHUNK * IW), np.int16)
    for c in range(NCHUNK):
        sub = arr[:, c * CH:(c + 1) * CH]         # [128, CH]
        L = sub.T.ravel()                         # L[g*128+p] = sub[p, g]
        w = L.reshape(IW, 16).T.astype(np.int16)  # [16, IW]
        out[:, c * IW:(c + 1) * IW] = np.tile(w, (8, 1))
    return out


def make_in_maps(inputs):
    """Full inputs -> per-core input maps + unshard metadata."""
    x = np.ascontiguousarray(np.asarray(inputs["x"], np.float32))
    row = np.asarray(inputs["row"])
    col = np.asarray(inputs["col"])
    values = np.asarray(inputs["values"], np.float32)
    noise = np.asarray(inputs["noise"], np.float32)
    batt = np.full(128, np.asarray(inputs["b_att"], np.float32).reshape(-1)[0]
                   - AOFF, np.float32)

    # global packing (per core) first, so col gidx can reference any core
    packs = []
    gpart = np.zeros(N, np.int64)
    gk = np.zeros(N, np.int64)
    for c in range(CORES):
        mask = row // NPC == c
        lr = row[mask] - c * NPC
        deg = np.bincount(lr, minlength=NPC)
        ne = int(mask.sum())
        part, kk = _pack_nodes(deg, ne)
        packs.append((part, kk, deg, ne))
        gpart[c * NPC:(c + 1) * NPC] = part
        gk[c * NPC:(c + 1) * NPC] = kk
    # packed global id of every original node
    gcore = np.arange(N) // NPC
    gidx_of_node = gcore * LNP + gpart * KMAX + gk     # [N], < FULLN

    in_maps = []
    perms = []
    for c in range(CORES):
        part, kk, deg, ne = packs[c]
        gsel = np.where(row // NPC == c)[0]
        lr = row[gsel] - c * NPC
        order = np.argsort(lr, kind="stable")
        eidx = gsel[order]
        lr = lr[order]

        # slot of each edge: partition part[lr], column = node slot start
        # + position within node
        node_first = np.searchsorted(lr, np.arange(NPC), side="left")
        pos_in_node = np.arange(ne) - node_first[lr]
        # node slot start within its partition: cumulative degree of
        # earlier nodes in the same partition
        deg64 = deg.astype(np.int64)
        csum = np.concatenate([[0], np.cumsum(deg64)])
        pstart_node = np.zeros(NPC, np.int64)  # first node id of partition
        first_in_part = np.ones(NPC, bool)
        first_in_part[1:] = part[1:] != part[:-1]
        pfirst = np.where(first_in_part)[0]
        pstart_of_part = np.zeros(128, np.int64)
        pstart_of_part[part[pfirst]] = csum[pfirst]
        node_slot_start = csum[:-1] - pstart_of_part[part]
        pslot = part[lr]
        jslot = node_slot_start[lr] + pos_in_node
        assert jslot.max() < F

        vals_s = np.zeros((128, F), np.float32)
        noise_s = np.full((128, F), 0.5, np.float32)
        seg = np.full((128, F), 127.0, np.float32)
        colg = np.zeros((128, F), np.int64)
        colsel = np.zeros((128, F), np.float32)
        perm = np.full((128, F), -1, np.int64)

        vals_s[pslot, jslot] = values[eidx]
        noise_s[pslot, jslot] = noise[eidx, 0]
        seg[pslot, jslot] = kk[lr].astype(np.float32)
        gc = gidx_of_node[col[eidx]]
        colg[pslot, jslot] = gc >> 1
        colsel[pslot, jslot] = (gc & 1).astype(np.float32)
        perm[pslot, jslot] = eidx

        # permuted x: xs[p*KMAX + k] = x[c*NPC + node]
        xs = np.zeros((LNP, D), np.float32)
        lid = gpart[c * NPC:(c + 1) * NPC] * KMAX + gk[c * NPC:(c + 1) * NPC]
        xs[lid] = x[c * NPC:(c + 1) * NPC]

        if BF16:
            import ml_dtypes
            xs = xs.astype(ml_dtypes.bfloat16)
            wnb_h = np.asarray(inputs["W_nb"],
                               np.float32).astype(ml_dtypes.bfloat16)
            wself_h = np.asarray(inputs["W_self"],
                                 np.float32).astype(ml_dtypes.bfloat16)
        else:
            wnb_h = np.asarray(inputs["W_nb"], np.float32)
            wself_h = np.asarray(inputs["W_self"], np.float32)
        in_maps.append({
            "x_shard": xs,
            "w_nb": wnb_h,
            "w_self": wself_h,
            "b_nb": np.asarray(inputs["b_nb"], np.float32),
            "b_self": np.asarray(inputs["b_self"], np.float32),
            "watt": np.asarray(inputs["W_att"], np.float32).reshape(-1),
            "batt": batt,
            "vals_s": vals_s,
            "noise_s": noise_s,
            "seg": seg,
            "colsel": colsel,
            "colidx16": _wrap16(colg),
        })
        perms.append(perm)
    return in_maps, perms


def unshard(results, perms):
    out = np.zeros(E, np.float32)
    for c in range(CORES):
        o = np.asarray(results[c]["out_s"])
        m = perms[c] >= 0
        out[perms[c][m]] = o[m]
    return out


# ======================================================================
# device program
# ======================================================================

def _build_body(tc):
    nc = tc.nc

    def din(name, shape, dtype=f32):
        return nc.dram_tensor(name, shape, dtype, kind="ExternalInput").ap()

    x_d = din("x_shard", [LNP, D], xdt)
    wnb_d = din("w_nb", [D, H], xdt)
    wself_d = din("w_self", [D, H], xdt)
    bnb_d = din("b_nb", [H])
    bself_d = din("b_self", [H])
    watt_d = din("watt", [2 * H])
    batt_d = din("batt", [128])
    vals_d = din("vals_s", [128, F])
    noise_d = din("noise_s", [128, F])
    seg_d = din("seg", [128, F])
    colsel_d = din("colsel", [128, F])
    colidx_d = din("colidx16", [128, NCHUNK * IW], i16)

    out_d = nc.dram_tensor("out_s", [128, F], f32, kind="ExternalOutput").ap()

    a1loc = nc.dram_tensor("a1loc", [LNP], f32).ap()
    a2loc = nc.dram_tensor("a2loc", [LNP], f32).ap()
    dinvloc = nc.dram_tensor("dinvloc", [LNP], f32).ap()
    a2full = nc.dram_tensor("a2full", [FULLN], f32, addr_space="Shared").ap()
    dinvfull = nc.dram_tensor("dinvfull", [FULLN], f32,
                              addr_space="Shared").ap()
    widea2 = nc.dram_tensor("widea2", [TROWS, ES], f32).ap()
    widedi = nc.dram_tensor("widedi", [TROWS, ES], f32).ap()

    groups = [list(range(CORES))]

    from contextlib import ExitStack
    ctx = _build_body.ctx
    const = ctx.enter_context(tc.tile_pool(name="const", bufs=1))
    meta = ctx.enter_context(tc.tile_pool(name="meta", bufs=1))
    p1 = ExitStack()
    xtp = p1.enter_context(tc.tile_pool(name="xt", bufs=1))
    hrp = p1.enter_context(tc.tile_pool(name="hr", bufs=1))
    mm_ps = p1.enter_context(tc.tile_pool(name="mm_ps", bufs=2, space="PSUM"))
    a_ps = p1.enter_context(tc.tile_pool(name="a_ps", bufs=1, space="PSUM"))

    # ---------------- constants ----------------
    w_sb = {}
    for nm, dram in (("nb", wnb_d), ("self", wself_d)):
        for k in range(2):
            t = const.tile([128, H], xdt, tag=f"w_{nm}{k}", name=f"w_{nm}{k}")
            nc.sync.dma_start(out=t[:], in_=dram[k * 128:(k + 1) * 128, :])
            w_sb[(nm, k)] = t
    b_sb = {}
    for nm, dram in (("nb", bnb_d), ("self", bself_d)):
        t = const.tile([128, 1], f32, tag=f"b_{nm}", name=f"b_{nm}")
        nc.sync.dma_start(out=t[:], in_=dram[:, None])
        b_sb[nm] = t
    wv_sb = {}
    for nm, sl in (("nb", slice(0, 128)), ("self", slice(128, 256))):
        tf = const.tile([128, 1], f32, tag=f"wvf_{nm}", name=f"wvf_{nm}")
        nc.sync.dma_start(out=tf[:], in_=watt_d[sl, None])
        t = const.tile([128, 1], xdt, tag=f"wv_{nm}", name=f"wv_{nm}")
        nc.vector.tensor_copy(out=t[:], in_=tf[:])
        wv_sb[nm] = t
    batt_sb = const.tile([128, 1], f32)
    nc.sync.dma_start(out=batt_sb[:], in_=batt_d[:, None])

    def constf(val, nm):
        t = const.tile([128, 1], f32, tag=f"c_{nm}", name=f"c_{nm}")
        nc.vector.memset(t[:], val)
        return t

    cb_dv = constf(DEBUG_VAR, "dv")
    cb_1mdv = constf(1.0 - DEBUG_VAR, "odv")
    cb_gamma = constf(GAMMA, "gm")

    # ---------------- phase 1: transpose x (XBAR DMA), MLP -------------
    xt = [xtp.tile([128, LNP], xdt, tag=f"xt{k}", name=f"xt{k}")
          for k in range(2)]
    for k in range(2):
        nc.sync.dma_start_transpose(
            out=xt[k][:], in_=x_d[:, k * 128:(k + 1) * 128])

    STRIP = 512
    strips = [(s, min(s + STRIP, LNP)) for s in range(0, LNP, STRIP)]
    a1w_store = None
    a2_store = None
    # "self" half first: a2 feeds the AllGather on the critical path
    for nm in ("self", "nb"):
        hr = hrp.tile([128, LNP], xdt, tag="hr", name=f"hr_{nm}")
        for s0, s1 in strips:
            ps = mm_ps.tile([128, STRIP], f32)
            for k in range(2):
                nc.tensor.matmul(
                    out=ps[:, :s1 - s0],
                    lhsT=w_sb[(nm, k)][:],
                    rhs=xt[k][:, s0:s1],
                    start=(k == 0),
                    stop=(k == 1),
                )
            nc.scalar.activation(
                out=hr[:, s0:s1], in_=ps[:, :s1 - s0], func=AF.Relu,
                bias=b_sb[nm][:, 0:1],
            )
        a_sb = meta.tile([1, LNP], f32, tag="a_sb", name=f"a_sb_{nm}")
        for s0, s1 in strips:
            aps = a_ps.tile([1, STRIP], f32, tag="aps", bufs=2)
            nc.tensor.matmul(
                out=aps[:, :s1 - s0],
                lhsT=wv_sb[nm][:],
                rhs=hr[:, s0:s1],
                start=True, stop=True,
            )
            nc.scalar.activation(out=a_sb[:, s0:s1], in_=aps[:, :s1 - s0],
                                 func=AF.Copy)
        dst = a2loc if nm == "self" else a1loc
        st = nc.sync.dma_start(out=dst[None, :], in_=a_sb[:])
        if nm == "self":
            a2_store = st
        else:
            a1w_store = st

    p1.close()
    edge = ctx.enter_context(tc.tile_pool(name="edge", bufs=1))
    gat = ctx.enter_context(tc.tile_pool(name="gat", bufs=2))
    stp = ctx.enter_context(tc.tile_pool(name="stt", bufs=2))
    widp = ctx.enter_context(tc.tile_pool(name="wid", bufs=1))

    # ---------------- AllGather a2, widen to [TROWS, ES] ---------------
    cc_a2 = nc.gpsimd.collective_compute(
        "AllGather", OP.bypass, replica_groups=groups,
        ins=[a2loc], outs=[a2full],
    )
    add_dep_helper(cc_a2.ins, a2_store.ins)

    def widen(full_dram, wide_dram, dep_cc):
        """[FULLN] table -> [TROWS, ES] wide rows (32 copies per node)."""
        WCH = 2
        KC = (FULLN // 128) // WCH       # table cols per chunk
        sts = []
        src = full_dram.rearrange("(p k) -> p k", p=128)
        dst = wide_dram.rearrange("(p f) e -> p (f e)", p=128)
        for i in range(WCH):
            a = widp.tile([128, KC], f32, tag="wa")
            ld = nc.sync.dma_start(out=a[:],
                                   in_=src[:, i * KC:(i + 1) * KC])
            add_dep_helper(ld.ins, dep_cc.ins)
            w = widp.tile([128, KC * 32], f32, tag="ww")
            for e in range(32):
                nc.vector.tensor_copy(
                    out=w[:].rearrange("p (k e) -> p k e", e=32)[:, :, e],
                    in_=a[:],
                )
            st = nc.sync.dma_start(
                out=dst[:, i * KC * 32:(i + 1) * KC * 32], in_=w[:]
            )
            sts.append(st)
        return sts

    wst_a2 = widen(a2full, widea2, cc_a2)

    # ---------------- edge inputs ----------------
    vals = edge.tile([128, F], f32)
    nc.sync.dma_start(out=vals[:], in_=vals_d[:])
    noise = edge.tile([128, F], f32)
    nc.sync.dma_start(out=noise[:], in_=noise_d[:])
    seg_sb = edge.tile([128, F], f32)
    nc.sync.dma_start(out=seg_sb[:], in_=seg_d[:])
    colsel = edge.tile([128, F], f32)
    nc.sync.dma_start(out=colsel[:], in_=colsel_d[:])
    colidx = edge.tile([128, NCHUNK * IW], i16)
    nc.sync.dma_start(out=colidx[:], in_=colidx_d[:])

    # noise logit
    lnu = edge.tile([128, F], f32)
    nc.scalar.activation(out=lnu[:], in_=noise[:], func=AF.Ln,
                         bias=cb_dv[:, 0:1], scale=1.0)
    nl = edge.tile([128, F], f32)
    nc.scalar.activation(out=nl[:], in_=noise[:], func=AF.Ln,
                         bias=cb_1mdv[:, 0:1], scale=-1.0)
    nc.vector.tensor_sub(nl[:], lnu[:], nl[:])

    # ---------------- a1 row expansion (select-accumulate) -------------
    # nva1[p, k] = a1(node p*64+k) + AOFF  (a1loc layout IS packed order)
    nva1 = meta.tile([128, KMAX], f32)
    lda1 = nc.sync.dma_start(
        out=nva1[:], in_=a1loc.rearrange("(p k) -> p k", k=KMAX))
    add_dep_helper(lda1.ins, a1w_store.ins)
    nva1o = meta.tile([128, KMAX], f32)
    nc.vector.tensor_scalar_add(nva1o[:], nva1[:], AOFF)

    def expand_rows_into(acc, nv, op_tag):
        """acc[p, j] += nv[p, seg[p, j]] via per-segment select-accumulate."""
        for k in range(KMAX):
            tmp = stp.tile([128, F], f32, tag="stt")
            nc.vector.scalar_tensor_tensor(
                out=tmp[:], in0=seg_sb[:], scalar=float(k),
                in1=nv[:, k:k + 1].to_broadcast((128, F)),
                op0=OP.is_equal, op1=OP.mult,
            )
            nc.vector.tensor_add(acc[:], acc[:], tmp[:])

    expand_rows_into(nl, nva1o, "a1")   # nl += a1[row] + AOFF

    # ---------------- a2 col gather (chunked dma_gather) ---------------
    def gather_add(wide_dram, dep_sts, dst, mode):
        """dst[:, chunk] (+)= table[col] per chunk; mode 'add' or 'set'."""
        for cchunk in range(NCHUNK):
            g = gat.tile([128, CH, ES], f32, tag="g")
            gi = nc.gpsimd.dma_gather(
                g[:], wide_dram[:, :],
                colidx[:, cchunk * IW:(cchunk + 1) * IW],
                num_idxs=NI, num_idxs_reg=NI, elem_size=ES,
                single_packet=False,
            )
            for dep_st in dep_sts:
                add_dep_helper(gi.ins, dep_st.ins)
            sl = slice(cchunk * CH, (cchunk + 1) * CH)
            diff = gat.tile([128, CH], f32, tag="d")
            nc.vector.tensor_sub(diff[:], g[:, :, 32], g[:, :, 0])
            selv = gat.tile([128, CH], f32, tag="s")
            nc.vector.scalar_tensor_tensor(
                out=selv[:], in0=colsel[:, sl], scalar=1.0,
                in1=diff[:], op0=OP.mult, op1=OP.mult,
            )
            nc.vector.tensor_add(selv[:], selv[:], g[:, :, 0])
            if mode == "add":
                nc.vector.tensor_add(dst[:, sl], dst[:, sl], selv[:])
            else:
                nc.vector.tensor_copy(out=dst[:, sl], in_=selv[:])

    gather_add(widea2, wst_a2, nl, "add")   # nl += a2[col]

    # ---------------- mask & masked values -----------------------------
    gate = edge.tile([128, F], f32)
    nc.scalar.activation(out=gate[:], in_=nl[:], func=AF.Sigmoid,
                         bias=batt_sb[:, 0:1])
    nc.scalar.activation(out=gate[:], in_=gate[:], func=AF.Relu,
                         bias=cb_gamma[:, 0:1], scale=ZETA - GAMMA)
    nc.vector.tensor_scalar_min(gate[:], gate[:], 1.0)
    mv = edge.tile([128, F], f32)
    nc.vector.tensor_mul(mv[:], vals[:], gate[:])

    # ---------------- rowsum (select-reduce) ---------------------------
    rowsum = meta.tile([128, KMAX], f32)
    for k in range(KMAX):
        tmp = stp.tile([128, F], f32, tag="stt")
        nc.vector.scalar_tensor_tensor(
            out=tmp[:], in0=seg_sb[:], scalar=float(k),
            in1=mv[:], op0=OP.is_equal, op1=OP.mult,
        )
        trash = stp.tile([128, F], f32, tag="act_trash")
        nc.scalar.activation(out=trash[:], in_=tmp[:], func=AF.Copy,
                             accum_out=rowsum[:, k:k + 1])
    rsp = meta.tile([128, KMAX], f32)
    nc.vector.tensor_scalar_add(rsp[:], rowsum[:], 1e-10)
    rcp = meta.tile([128, KMAX], f32)
    nc.vector.reciprocal(rcp[:], rsp[:])
    dinv = meta.tile([128, KMAX], f32)
    nc.scalar.activation(out=dinv[:], in_=rcp[:], func=AF.Sqrt)
    wdl = nc.sync.dma_start(
        out=dinvloc.rearrange("(p k) -> p k", k=KMAX), in_=dinv[:]
    )

    # ---------------- AllGather d_inv + widen + final gathers ----------
    cc_di = nc.gpsimd.collective_compute(
        "AllGather", OP.bypass, replica_groups=groups,
        ins=[dinvloc], outs=[dinvfull],
    )
    add_dep_helper(cc_di.ins, wdl.ins)
    wst_di = widen(dinvfull, widedi, cc_di)

    # dinv row expansion
    drow = edge.tile([128, F], f32)
    nc.vector.memset(drow[:], 0.0)
    expand_rows_into(drow, dinv, "di")
    nc.vector.tensor_mul(drow[:], mv[:], drow[:])

    # dinv col gather
    dcol = edge.tile([128, F], f32)
    gather_add(widedi, wst_di, dcol, "set")

    nc.vector.tensor_mul(dcol[:], drow[:], dcol[:])
    nc.sync.dma_start(out=out_d[:], in_=dcol[:])


@functools.lru_cache(maxsize=1)
def build_nc():
    from contextlib import ExitStack
    nc = bacc.Bacc(
        "TRN2", target_bir_lowering=False, debug=False, num_devices=CORES
    )
    with tile.TileContext(nc) as tc:
        with ExitStack() as ctx:
            _build_body.ctx = ctx
            _build_body(tc)
    nc.compile()
    return nc


# ======================================================================
# entry point
# ======================================================================

def kernel(**inputs) -> np.ndarray:
    in_maps, perms = make_in_maps(inputs)
    nc = build_nc()
    res = run_bass_kernel_spmd(nc, in_maps, core_ids=list(range(CORES)))
    return unshard(res.results, perms)


if __name__ == "__main__":
    import reference as ref_mod
    inputs = {k: np.asarray(v) for k, v in ref_mod.setup_inputs().items()}
    expected = np.asarray(ref_mod.reference(**inputs))
    actual = kernel(**inputs)
    rel = np.linalg.norm(actual - expected) / np.linalg.norm(expected)
    print("Relative error:", rel)


# revision 3
# speedup vs baseline: 1.0683x; 1.0683x over previous
"""Trainium2 Bass kernel for nn_DenoisingNet (GNN message passing), v2.

Strategy
--------
The per-edge attention MLP decomposes into per-node scalars:
    log_alpha[e] = a1[row[e]] + a2[col[e]] + b_att
so the MLP runs over N=50k nodes instead of E=800k edges.

Sharding: edges bucketed by row-range (core c owns rows [c*6250,
(c+1)*6250)), row-sorted, whole nodes packed per SBUF partition with a
64-node/partition cap.  Nodes are RE-INDEXED device-side in packed
order: node (p, k) -> local id p*64+k (host permutes x rows), so node
tables are plain [128, 64] SBUF tiles / [8192] DRAM vectors with static
access patterns - no boundary gathers, no scan.

Per-edge col gathers use the MoE `dma_gather` ucode (single_packet=False;
the single-packet path dies above ~1k indices): global node tables are
AllGathered to [65536], widened on-device to [32768, 64] f32 rows (256 B,
two nodes per row, 32 copies each), gathered per 6912-index chunk with
wrapped int16 indices, and a predicated select picks the odd/even half.

Row-side expansions (a1[row], dinv[row]) and the per-node rowsum use
KMAX=64 select-accumulate / select-reduce passes over the [128, F] slot
grid (seg[p,j] = k of the owning node, constant per contiguous run).
"""

import functools

import numpy as np

import concourse.bacc as bacc
import concourse.tile as tile
from concourse import mybir
from concourse.bass_utils import run_bass_kernel_spmd
from concourse.tile import add_dep_helper

# ---- problem constants (hardcoded per contest contract) ----
N = 50000
E = 800000
D = 256
H = 128
CORES = 8
NPC = N // CORES          # 6250 nodes per core
KMAX = 64                 # max packed nodes per edge-partition
LNP = 128 * KMAX          # 8192: local node table length (packed ids)
NT = LNP // 128           # 64 node tiles
FULLN = CORES * LNP       # 65536 global table length
TROWS = FULLN // 2        # 32768 wide-table rows (2 nodes/row)
F = 848                   # edge slots per partition (max pack load 836)
CH = 53                   # gather chunk columns (2 in-flight chunks stay
                          # under the 1024-desc/lane SWDGE ring)
NCHUNK = F // CH          # chunks (F must be divisible by CH)
NI = 128 * CH             # indices per chunk
IW = NI // 16             # wrapped-idx columns per chunk
ES = 64                   # wide-table row: 64 f32 = 256 B
BF16 = 1

GAMMA = -0.5
ZETA = 1.1
DEBUG_VAR = 1e-07
AOFF = 64.0               # positivity offset for the a1 expansion

f32 = mybir.dt.float32
bf16 = mybir.dt.bfloat16
i32 = mybir.dt.int32
i16 = mybir.dt.int16
xdt = bf16 if BF16 else f32
AF = mybir.ActivationFunctionType
OP = mybir.AluOpType


# ======================================================================
# host-side sharding
# ======================================================================

def _pack_nodes(deg, ne):
    """Sequential greedy pack of nodes (in id order) into 128 partitions.

    Returns part_of_node, k_of_node."""
    part = np.zeros(NPC, np.int32)
    kk = np.zeros(NPC, np.int32)
    p = 0
    slots_p = 0
    k_p = 0
    placed = 0
    target = -(-ne // 128)
    for l in range(NPC):
        d = int(deg[l])
        if p < 127 and (slots_p + d > target or k_p >= KMAX):
            p += 1
            slots_p = 0
            k_p = 0
            target = -(-(ne - placed) // (128 - p))
        assert slots_p + d <= F, f"partition overflow {slots_p}+{d}"
        part[l] = p
        kk[l] = k_p
        slots_p += d
        k_p += 1
        placed += d
    return part, kk


def _wrap16(arr):
    """[128, F] idx grid -> wrapped int16 [128, NCHUNK*IW] for dma_gather."""
    out = np.empty((128, NCHUNK * IW), np.int16)
    for c in range(NCHUNK):
        sub = arr[:, c * CH:(c + 1) * CH]         # [128, CH]
        L = sub.T.ravel()                         # L[g*128+p] = sub[p, g]
        w = L.reshape(IW, 16).T.astype(np.int16)  # [16, IW]
        out[:, c * IW:(c + 1) * IW] = np.tile(w, (8, 1))
    return out


def make_in_maps(inputs):
    """Full inputs -> per-core input maps + unshard metadata."""
    x = np.ascontiguousarray(np.asarray(inputs["x"], np.float32))
    row = np.asarray(inputs["row"])
    col = np.asarray(inputs["col"])
    values = np.asarray(inputs["values"], np.float32)
    noise = np.asarray(inputs["noise"], np.float32)
    batt = np.full(128, np.asarray(inputs["b_att"], np.float32).reshape(-1)[0]
                   - AOFF, np.float32)

    # global packing (per core) first, so col gidx can reference any core
    packs = []
    gpart = np.zeros(N, np.int64)
    gk = np.zeros(N, np.int64)
    for c in range(CORES):
        mask = row // NPC == c
        lr = row[mask] - c * NPC
        deg = np.bincount(lr, minlength=NPC)
        ne = int(mask.sum())
        part, kk = _pack_nodes(deg, ne)
        packs.append((part, kk, deg, ne))
        gpart[c * NPC:(c + 1) * NPC] = part
        gk[c * NPC:(c + 1) * NPC] = kk
    # packed global id of every original node
    gcore = np.arange(N) // NPC
    gidx_of_node = gcore * LNP + gpart * KMAX + gk     # [N], < FULLN

    in_maps = []
    perms = []
    for c in range(CORES):
        part, kk, deg, ne = packs[c]
        gsel = np.where(row // NPC == c)[0]
        lr = row[gsel] - c * NPC
        order = np.argsort(lr, kind="stable")
        eidx = gsel[order]
        lr = lr[order]

        # slot of each edge: partition part[lr], column = node slot start
        # + position within node
        node_first = np.searchsorted(lr, np.arange(NPC), side="left")
        pos_in_node = np.arange(ne) - node_first[lr]
        # node slot start within its partition: cumulative degree of
        # earlier nodes in the same partition
        deg64 = deg.astype(np.int64)
        csum = np.concatenate([[0], np.cumsum(deg64)])
        pstart_node = np.zeros(NPC, np.int64)  # first node id of partition
        first_in_part = np.ones(NPC, bool)
        first_in_part[1:] = part[1:] != part[:-1]
        pfirst = np.where(first_in_part)[0]
        pstart_of_part = np.zeros(128, np.int64)
        pstart_of_part[part[pfirst]] = csum[pfirst]
        node_slot_start = csum[:-1] - pstart_of_part[part]
        pslot = part[lr]
        jslot = node_slot_start[lr] + pos_in_node
        assert jslot.max() < F

        vals_s = np.zeros((128, F), np.float32)
        noise_s = np.full((128, F), 0.5, np.float32)
        seg = np.full((128, F), 127.0, np.float32)
        colg = np.zeros((128, F), np.int64)
        colsel = np.zeros((128, F), np.float32)
        perm = np.full((128, F), -1, np.int64)

        vals_s[pslot, jslot] = values[eidx]
        noise_s[pslot, jslot] = noise[eidx, 0]
        seg[pslot, jslot] = kk[lr].astype(np.float32)
        gc = gidx_of_node[col[eidx]]
        colg[pslot, jslot] = gc >> 1
        colsel[pslot, jslot] = (gc & 1).astype(np.float32)
        perm[pslot, jslot] = eidx

        # permuted x: xs[p*KMAX + k] = x[c*NPC + node]
        xs = np.zeros((LNP, D), np.float32)
        lid = gpart[c * NPC:(c + 1) * NPC] * KMAX + gk[c * NPC:(c + 1) * NPC]
        xs[lid] = x[c * NPC:(c + 1) * NPC]

        if BF16:
            import ml_dtypes
            xs = xs.astype(ml_dtypes.bfloat16)
            wnb_h = np.asarray(inputs["W_nb"],
                               np.float32).astype(ml_dtypes.bfloat16)
            wself_h = np.asarray(inputs["W_self"],
                                 np.float32).astype(ml_dtypes.bfloat16)
        else:
            wnb_h = np.asarray(inputs["W_nb"], np.float32)
            wself_h = np.asarray(inputs["W_self"], np.float32)
        in_maps.append({
            "x_shard": xs,
            "w_nb": wnb_h,
            "w_self": wself_h,
            "b_nb": np.asarray(inputs["b_nb"], np.float32),
            "b_self": np.asarray(inputs["b_self"], np.float32),
            "watt": np.asarray(inputs["W_att"], np.float32).reshape(-1),
            "batt": batt,
            "vals_s": vals_s,
            "noise_s": noise_s,
            "seg": seg,
            "colsel": colsel,
            "colidx16": _wrap16(colg),
        })
        perms.append(perm)
    return in_maps, perms


def unshard(results, perms):
    out = np.zeros(E, np.float32)
    for c in range(CORES):
        o = np.asarray(results[c]["out_s"])
        m = perms[c] >= 0
        out[perms[c][m]] = o[m]
    return out


# ======================================================================
# device program
# ======================================================================

def _build_body(tc):
    nc = tc.nc

    def din(name, shape, dtype=f32):
        return nc.dram_tensor(name, shape, dtype, kind="ExternalInput").ap()

    x_d = din("x_shard", [LNP, D], xdt)
    wnb_d = din("w_nb", [D, H], xdt)
    wself_d = din("w_self", [D, H], xdt)
    bnb_d = din("b_nb", [H])
    bself_d = din("b_self", [H])
    watt_d = din("watt", [2 * H])
    batt_d = din("batt", [128])
    vals_d = din("vals_s", [128, F])
    noise_d = din("noise_s", [128, F])
    seg_d = din("seg", [128, F])
    colsel_d = din("colsel", [128, F])
    colidx_d = din("colidx16", [128, NCHUNK * IW], i16)

    out_d = nc.dram_tensor("out_s", [128, F], f32, kind="ExternalOutput").ap()

    a1loc = nc.dram_tensor("a1loc", [LNP], f32).ap()
    a2loc = nc.dram_tensor("a2loc", [LNP], f32).ap()
    dinvloc = nc.dram_tensor("dinvloc", [LNP], f32).ap()
    a2full = nc.dram_tensor("a2full", [FULLN], f32, addr_space="Shared").ap()
    dinvfull = nc.dram_tensor("dinvfull", [FULLN], f32,
                              addr_space="Shared").ap()
    widea2 = nc.dram_tensor("widea2", [TROWS, ES], f32).ap()
    widedi = nc.dram_tensor("widedi", [TROWS, ES], f32).ap()

    groups = [list(range(CORES))]

    from contextlib import ExitStack
    ctx = _build_body.ctx
    const = ctx.enter_context(tc.tile_pool(name="const", bufs=1))
    meta = ctx.enter_context(tc.tile_pool(name="meta", bufs=1))
    p1 = ExitStack()
    xtp = p1.enter_context(tc.tile_pool(name="xt", bufs=1))
    hrp = p1.enter_context(tc.tile_pool(name="hr", bufs=1))
    mm_ps = p1.enter_context(tc.tile_pool(name="mm_ps", bufs=2, space="PSUM"))
    a_ps = p1.enter_context(tc.tile_pool(name="a_ps", bufs=1, space="PSUM"))

    # ---------------- constants ----------------
    w_sb = {}
    for nm, dram in (("nb", wnb_d), ("self", wself_d)):
        for k in range(2):
            t = const.tile([128, H], xdt, tag=f"w_{nm}{k}", name=f"w_{nm}{k}")
            nc.sync.dma_start(out=t[:], in_=dram[k * 128:(k + 1) * 128, :])
            w_sb[(nm, k)] = t
    b_sb = {}
    for nm, dram in (("nb", bnb_d), ("self", bself_d)):
        t = const.tile([128, 1], f32, tag=f"b_{nm}", name=f"b_{nm}")
        nc.sync.dma_start(out=t[:], in_=dram[:, None])
        b_sb[nm] = t
    wv_sb = {}
    for nm, sl in (("nb", slice(0, 128)), ("self", slice(128, 256))):
        tf = const.tile([128, 1], f32, tag=f"wvf_{nm}", name=f"wvf_{nm}")
        nc.sync.dma_start(out=tf[:], in_=watt_d[sl, None])
        t = const.tile([128, 1], xdt, tag=f"wv_{nm}", name=f"wv_{nm}")
        nc.vector.tensor_copy(out=t[:], in_=tf[:])
        wv_sb[nm] = t
    batt_sb = const.tile([128, 1], f32)
    nc.sync.dma_start(out=batt_sb[:], in_=batt_d[:, None])

    def constf(val, nm):
        t = const.tile([128, 1], f32, tag=f"c_{nm}", name=f"c_{nm}")
        nc.vector.memset(t[:], val)
        return t

    cb_dv = constf(DEBUG_VAR, "dv")
    cb_1mdv = constf(1.0 - DEBUG_VAR, "odv")
    cb_gamma = constf(GAMMA, "gm")

    # ---------------- phase 1: transpose x (XBAR DMA), MLP -------------
    xt = [xtp.tile([128, LNP], xdt, tag=f"xt{k}", name=f"xt{k}")
          for k in range(2)]
    TCH = LNP // 4
    for i in range(4):
        for k in range(2):
            nc.sync.dma_start_transpose(
                out=xt[k][:, i * TCH:(i + 1) * TCH],
                in_=x_d[i * TCH:(i + 1) * TCH, k * 128:(k + 1) * 128])

    STRIP = 512
    strips = [(s, min(s + STRIP, LNP)) for s in range(0, LNP, STRIP)]
    a1w_store = None
    a2_store = None
    # "self" half first: a2 feeds the AllGather on the critical path
    for nm in ("self", "nb"):
        hr = hrp.tile([128, LNP], xdt, tag="hr", name=f"hr_{nm}")
        for s0, s1 in strips:
            ps = mm_ps.tile([128, STRIP], f32)
            for k in range(2):
                nc.tensor.matmul(
                    out=ps[:, :s1 - s0],
                    lhsT=w_sb[(nm, k)][:],
                    rhs=xt[k][:, s0:s1],
                    start=(k == 0),
                    stop=(k == 1),
                )
            nc.scalar.activation(
                out=hr[:, s0:s1], in_=ps[:, :s1 - s0], func=AF.Relu,
                bias=b_sb[nm][:, 0:1],
            )
        a_sb = meta.tile([1, LNP], f32, tag="a_sb", name=f"a_sb_{nm}")
        for s0, s1 in strips:
            aps = a_ps.tile([1, STRIP], f32, tag="aps", bufs=2)
            nc.tensor.matmul(
                out=aps[:, :s1 - s0],
                lhsT=wv_sb[nm][:],
                rhs=hr[:, s0:s1],
                start=True, stop=True,
            )
            nc.scalar.activation(out=a_sb[:, s0:s1], in_=aps[:, :s1 - s0],
                                 func=AF.Copy)
        dst = a2loc if nm == "self" else a1loc
        st = nc.sync.dma_start(out=dst[None, :], in_=a_sb[:])
        if nm == "self":
            a2_store = st
        else:
            a1w_store = st

    p1.close()
    edge = ctx.enter_context(tc.tile_pool(name="edge", bufs=1))
    gat = ctx.enter_context(tc.tile_pool(name="gat", bufs=2))
    stp = ctx.enter_context(tc.tile_pool(name="stt", bufs=2))
    widp = ctx.enter_context(tc.tile_pool(name="wid", bufs=2))

    # ---------------- AllGather a2, widen to [TROWS, ES] ---------------
    cc_a2 = nc.gpsimd.collective_compute(
        "AllGather", OP.bypass, replica_groups=groups,
        ins=[a2loc], outs=[a2full],
    )
    add_dep_helper(cc_a2.ins, a2_store.ins)

    def widen(full_dram, wide_dram, dep_cc):
        """[FULLN] table -> [TROWS, ES] wide rows (32 copies per node)."""
        WCH = 4
        KC = (FULLN // 128) // WCH       # table cols per chunk
        sts = []
        src = full_dram.rearrange("(p k) -> p k", p=128)
        dst = wide_dram.rearrange("(p f) e -> p (f e)", p=128)
        for i in range(WCH):
            a = widp.tile([128, KC], f32, tag="wa")
            ld = nc.sync.dma_start(out=a[:],
                                   in_=src[:, i * KC:(i + 1) * KC])
            add_dep_helper(ld.ins, dep_cc.ins)
            w = widp.tile([128, KC * 32], f32, tag="ww")
            for e in range(32):
                nc.vector.tensor_copy(
                    out=w[:].rearrange("p (k e) -> p k e", e=32)[:, :, e],
                    in_=a[:],
                )
            st = nc.sync.dma_start(
                out=dst[:, i * KC * 32:(i + 1) * KC * 32], in_=w[:]
            )
            sts.append(st)
        return sts

    wst_a2 = widen(a2full, widea2, cc_a2)

    # ---------------- edge inputs ----------------
    vals = edge.tile([128, F], f32)
    nc.sync.dma_start(out=vals[:], in_=vals_d[:])
    noise = edge.tile([128, F], f32)
    nc.sync.dma_start(out=noise[:], in_=noise_d[:])
    seg_sb = edge.tile([128, F], f32)
    nc.sync.dma_start(out=seg_sb[:], in_=seg_d[:])
    colsel = edge.tile([128, F], f32)
    nc.sync.dma_start(out=colsel[:], in_=colsel_d[:])
    colidx = edge.tile([128, NCHUNK * IW], i16)
    nc.sync.dma_start(out=colidx[:], in_=colidx_d[:])

    # noise logit
    lnu = edge.tile([128, F], f32)
    nc.scalar.activation(out=lnu[:], in_=noise[:], func=AF.Ln,
                         bias=cb_dv[:, 0:1], scale=1.0)
    nl = edge.tile([128, F], f32)
    nc.scalar.activation(out=nl[:], in_=noise[:], func=AF.Ln,
                         bias=cb_1mdv[:, 0:1], scale=-1.0)
    nc.vector.tensor_sub(nl[:], lnu[:], nl[:])

    # ---------------- a1 row expansion (select-accumulate) -------------
    # nva1[p, k] = a1(node p*64+k) + AOFF  (a1loc layout IS packed order)
    nva1 = meta.tile([128, KMAX], f32)
    lda1 = nc.sync.dma_start(
        out=nva1[:], in_=a1loc.rearrange("(p k) -> p k", k=KMAX))
    add_dep_helper(lda1.ins, a1w_store.ins)
    nva1o = meta.tile([128, KMAX], f32)
    nc.vector.tensor_scalar_add(nva1o[:], nva1[:], AOFF)

    def expand_rows_into(acc, nv, op_tag):
        """acc[p, j] += nv[p, seg[p, j]] via per-segment select-accumulate."""
        for k in range(KMAX):
            tmp = stp.tile([128, F], f32, tag="stt")
            nc.vector.scalar_tensor_tensor(
                out=tmp[:], in0=seg_sb[:], scalar=float(k),
                in1=nv[:, k:k + 1].to_broadcast((128, F)),
                op0=OP.is_equal, op1=OP.mult,
            )
            nc.vector.tensor_add(acc[:], acc[:], tmp[:])

    expand_rows_into(nl, nva1o, "a1")   # nl += a1[row] + AOFF

    # ---------------- a2 col gather (chunked dma_gather) ---------------
    def gather_add(wide_dram, dep_sts, dst, mode):
        """dst[:, chunk] (+)= table[col] per chunk; mode 'add' or 'set'."""
        for cchunk in range(NCHUNK):
            g = gat.tile([128, CH, ES], f32, tag="g")
            gi = nc.gpsimd.dma_gather(
                g[:], wide_dram[:, :],
                colidx[:, cchunk * IW:(cchunk + 1) * IW],
                num_idxs=NI, num_idxs_reg=NI, elem_size=ES,
                single_packet=False,
            )
            for dep_st in dep_sts:
                add_dep_helper(gi.ins, dep_st.ins)
            sl = slice(cchunk * CH, (cchunk + 1) * CH)
            diff = gat.tile([128, CH], f32, tag="d")
            nc.vector.tensor_sub(diff[:], g[:, :, 32], g[:, :, 0])
            selv = gat.tile([128, CH], f32, tag="s")
            nc.vector.scalar_tensor_tensor(
                out=selv[:], in0=colsel[:, sl], scalar=1.0,
                in1=diff[:], op0=OP.mult, op1=OP.mult,
            )
            nc.vector.tensor_add(selv[:], selv[:], g[:, :, 0])
            if mode == "add":
                nc.vector.tensor_add(dst[:, sl], dst[:, sl], selv[:])
            else:
                nc.vector.tensor_copy(out=dst[:, sl], in_=selv[:])

    gather_add(widea2, wst_a2, nl, "add")   # nl += a2[col]

    # ---------------- mask & masked values -----------------------------
    gate = edge.tile([128, F], f32)
    nc.scalar.activation(out=gate[:], in_=nl[:], func=AF.Sigmoid,
                         bias=batt_sb[:, 0:1])
    nc.scalar.activation(out=gate[:], in_=gate[:], func=AF.Relu,
                         bias=cb_gamma[:, 0:1], scale=ZETA - GAMMA)
    nc.vector.tensor_scalar_min(gate[:], gate[:], 1.0)
    mv = edge.tile([128, F], f32)
    nc.vector.tensor_mul(mv[:], vals[:], gate[:])

    # ---------------- rowsum (select-reduce) ---------------------------
    rowsum = meta.tile([128, KMAX], f32)
    for k in range(KMAX):
        tmp = stp.tile([128, F], f32, tag="stt")
        nc.vector.scalar_tensor_tensor(
            out=tmp[:], in0=seg_sb[:], scalar=float(k),
            in1=mv[:], op0=OP.is_equal, op1=OP.mult,
        )
        trash = stp.tile([128, F], f32, tag="act_trash")
        nc.scalar.activation(out=trash[:], in_=tmp[:], func=AF.Copy,
                             accum_out=rowsum[:, k:k + 1])
    rsp = meta.tile([128, KMAX], f32)
    nc.vector.tensor_scalar_add(rsp[:], rowsum[:], 1e-10)
    rcp = meta.tile([128, KMAX], f32)
    nc.vector.reciprocal(rcp[:], rsp[:])
    dinv = meta.tile([128, KMAX], f32)
    nc.scalar.activation(out=dinv[:], in_=rcp[:], func=AF.Sqrt)
    wdl = nc.sync.dma_start(
        out=dinvloc.rearrange("(p k) -> p k", k=KMAX), in_=dinv[:]
    )

    # ---------------- AllGather d_inv + widen + final gathers ----------
    cc_di = nc.gpsimd.collective_compute(
        "AllGather", OP.bypass, replica_groups=groups,
        ins=[dinvloc], outs=[dinvfull],
    )
    add_dep_helper(cc_di.ins, wdl.ins)
    wst_di = widen(dinvfull, widedi, cc_di)

    # dinv row expansion
    drow = edge.tile([128, F], f32)
    nc.vector.memset(drow[:], 0.0)
    expand_rows_into(drow, dinv, "di")
    nc.vector.tensor_mul(drow[:], mv[:], drow[:])

    # dinv col gather
    dcol = edge.tile([128, F], f32)
    gather_add(widedi, wst_di, dcol, "set")

    nc.vector.tensor_mul(dcol[:], drow[:], dcol[:])
    nc.sync.dma_start(out=out_d[:], in_=dcol[:])


@functools.lru_cache(maxsize=1)
def build_nc():
    from contextlib import ExitStack
    nc = bacc.Bacc(
        "TRN2", target_bir_lowering=False, debug=False, num_devices=CORES
    )
    with tile.TileContext(nc) as tc:
        with ExitStack() as ctx:
            _build_body.ctx = ctx
            _build_body(tc)
    nc.compile()
    return nc


# ======================================================================
# entry point
# ======================================================================

def kernel(**inputs) -> np.ndarray:
    in_maps, perms = make_in_maps(inputs)
    nc = build_nc()
    res = run_bass_kernel_spmd(nc, in_maps, core_ids=list(range(CORES)))
    return unshard(res.results, perms)


if __name__ == "__main__":
    import reference as ref_mod
    inputs = {k: np.asarray(v) for k, v in ref_mod.setup_inputs().items()}
    expected = np.asarray(ref_mod.reference(**inputs))
    actual = kernel(**inputs)
    rel = np.linalg.norm(actual - expected) / np.linalg.norm(expected)
    print("Relative error:", rel)
